# revision 1
# baseline (speedup 1.0000x reference)
"""Trainium2 Bass kernel for nn_Attention_90752658965090.

Computes, per batch element b of x[16, 512, 64, 64]:
  xn   = GroupNorm(8 groups, eps=1e-5, affine)(x[b])            # [512, 4096]
  q,k,v = split(qkv_w @ xn + qkv_b)                             # each [512, 4096]
  attn = softmax((q @ k^T) * 512**-0.5, axis=-1)                # [512, 512]
  out  = proj_w @ (attn @ v) + proj_b                           # [512, 4096]
  y[b] = x[b] + out

Sharding: data-parallel over batch, 2 batch elements per NeuronCore on 8 cores.
Matmuls run in float32r (full-rate PE, ~12-bit mantissa, fp32 accumulate).
"""
import sys

sys.path.insert(0, "/opt/trn_rl_repo")

import numpy as np

import concourse.bass as bass
import concourse.mybir as mybir
import concourse.tile as tile
from concourse import bacc

B, C, HW = 16, 512, 4096
NCORES = 8
BPC = B // NCORES          # batches per core
P = 128
CT = C // P                # 4 c-tiles
NCH = HW // 512            # 8 n-chunks of 512
GROUPS = 8
EPS = 1e-5
INV_N = 1.0 / (C // GROUPS)   # bn_aggr already normalizes over HW; group-combine over 64 channels
SCALE = float(C) ** -0.5

F32 = mybir.dt.float32
F32R = mybir.dt.float32r
AX = mybir.AxisListType
OP = mybir.AluOpType
AF = mybir.ActivationFunctionType


def build_program(repeat=1):
    nc = bacc.Bacc("TRN2", target_bir_lowering=False, debug=False, num_devices=NCORES)

    x_d = nc.dram_tensor("x", [BPC, C, HW], F32, kind="ExternalInput")
    y_d = nc.dram_tensor("y", [BPC, C, HW], F32, kind="ExternalOutput")
    wqkvT_d = nc.dram_tensor("wqkvT", [C, 3 * C], F32R, kind="ExternalInput")
    wprojT_d = nc.dram_tensor("wprojT", [C, C], F32R, kind="ExternalInput")
    qkb_d = nc.dram_tensor("qkb", [P, 2 * C], F32, kind="ExternalInput")
    cols_d = nc.dram_tensor("cols", [P, 4 * CT], F32, kind="ExternalInput")
    indp_d = nc.dram_tensor("indp", [P, GROUPS * CT], F32, kind="ExternalInput")
    indT_d = nc.dram_tensor("indT", [GROUPS, C], F32, kind="ExternalInput")
    ident_d = nc.dram_tensor("ident", [P, P], F32, kind="ExternalInput")

    from contextlib import ExitStack
    with tile.TileContext(nc) as tc, ExitStack() as ctx:
        wgt = ctx.enter_context(tc.tile_pool(name="wgt", bufs=1))
        xin = ctx.enter_context(tc.tile_pool(name="xin", bufs=14))
        xnp = ctx.enter_context(tc.tile_pool(name="xnp", bufs=8))
        qkp = ctx.enter_context(tc.tile_pool(name="qkp", bufs=3))
        vch = ctx.enter_context(tc.tile_pool(name="vch", bufs=8))
        epool = ctx.enter_context(tc.tile_pool(name="epool", bufs=2 * CT))
        etp = ctx.enter_context(tc.tile_pool(name="etp", bufs=2 * CT))
        lgp = ctx.enter_context(tc.tile_pool(name="lgp", bufs=CT))
        aop = ctx.enter_context(tc.tile_pool(name="aop", bufs=2 * CT))
        yop = ctx.enter_context(tc.tile_pool(name="yop", bufs=8))
        sm = ctx.enter_context(tc.tile_pool(name="sm", bufs=2 * CT))
        psA = ctx.enter_context(tc.tile_pool(name="psA", bufs=CT, space=bass.MemorySpace.PSUM))
        psB = ctx.enter_context(tc.tile_pool(name="psB", bufs=4, space=bass.MemorySpace.PSUM))

        # --- load constants/weights ---
        def stats_chunks(b, t, st6, ch_lo, ch_hi):
            for ch in range(ch_lo, ch_hi):
                xt = xin.tile([P, 512], F32, tag="xin", name=f"xs{b}_{t}_{ch}")
                nc.sync.dma_start(
                    xt[:], x_d[b, t * P:(t + 1) * P, ch * 512:(ch + 1) * 512])
                nc.vector.bn_stats(st6[:, ch, :], xt[:])

        def stats_end(b, t, st6):
            mv = sm.tile([P, 2], F32, tag="mv", name=f"mv{b}_{t}")
            nc.vector.bn_aggr(mv[:], st6[:])
            # ms: col0 = mean, col1 = E[x^2] = var + mean^2
            ms = sm.tile([P, 2], F32, tag="ms", name=f"ms{b}_{t}")
            nc.vector.tensor_copy(ms[:, 0:1], mv[:, 0:1])
            nc.vector.scalar_tensor_tensor(
                out=ms[:, 1:2], in0=mv[:, 0:1], scalar=mv[:, 0:1],
                in1=mv[:, 1:2], op0=OP.mult, op1=OP.add)
            return ms

        def stats_tile(b, t):
            st6 = sm.tile([P, NCH, 6], F32, tag="st6", name=f"st6_{b}_{t}")
            stats_chunks(b, t, st6, 0, NCH)
            return stats_end(b, t, st6)

        cols = wgt.tile([P, 4 * CT], F32, tag="cols")
        nc.sync.dma_start(cols[:], cols_d[:])
        indp = wgt.tile([P, GROUPS * CT], F32, tag="indp")
        nc.sync.dma_start(indp[:], indp_d[:])
        indT8 = wgt.tile([GROUPS, C], F32, tag="indT8")
        nc.sync.dma_start(indT8[:], indT_d[:])
        nwc = [cols[:, 4 * t + 0:4 * t + 1] for t in range(CT)]
        nbc = [cols[:, 4 * t + 1:4 * t + 2] for t in range(CT)]
        vbc = [cols[:, 4 * t + 2:4 * t + 3] for t in range(CT)]
        pbc = [cols[:, 4 * t + 3:4 * t + 4] for t in range(CT)]
        indt = [indp[:, GROUPS * t:GROUPS * (t + 1)] for t in range(CT)]
        indTt = [indT8[:, t * P:(t + 1) * P] for t in range(CT)]
        # Interleave batch-0 stats reads with weight loads so the first Q
        # matmuls (which need all wqkvT tiles) start as early as DMA BW allows.
        wqkvT = []
        wprojT = []
        ms0 = [] if repeat == 1 else None
        for t in range(CT):
            if ms0 is not None:
                ms0.append(stats_tile(0, t))
            w1 = wgt.tile([P, 3 * C], F32R, tag=f"wqkv{t}")
            nc.sync.dma_start(w1[:], wqkvT_d[t * P:(t + 1) * P, :])
            wqkvT.append(w1)
        qkb = wgt.tile([P, 2 * C], F32, tag="qkb")
        nc.sync.dma_start(qkb[:], qkb_d[:])
        for t in range(CT):
            w2 = wgt.tile([P, C], F32R, tag=f"wproj{t}")
            nc.sync.dma_start(w2[:], wprojT_d[t * P:(t + 1) * P, :])
            wprojT.append(w2)
        ident = wgt.tile([P, P], F32, tag="ident")
        nc.sync.dma_start(ident[:], ident_d[:])
        eps_t = wgt.tile([GROUPS, 1], F32, tag="eps")
        nc.vector.memset(eps_t[:], EPS)

        from contextlib import nullcontext

        def stats_final(b, ms_tiles):
            gps = psA.tile([GROUPS, 2], F32, tag="attn", name=f"gps{b}")
            for t in range(CT):
                nc.tensor.matmul(gps[:], indt[t], ms_tiles[t][:],
                                 start=(t == 0), stop=(t == CT - 1))
            gsb = sm.tile([GROUPS, 2], F32, tag="gsb", name=f"gsb{b}")
            nc.scalar.activation(gsb[:], gps[:], AF.Copy, scale=INV_N)
            m2 = sm.tile([GROUPS, 1], F32, tag="m2", name=f"m2_{b}")
            nc.vector.tensor_tensor(m2[:], gsb[:, 0:1], gsb[:, 0:1], op=OP.mult)
            var = sm.tile([GROUPS, 1], F32, tag="var", name=f"var{b}")
            nc.vector.tensor_tensor(var[:], gsb[:, 1:2], m2[:], op=OP.subtract)
            sq = sm.tile([GROUPS, 1], F32, tag="sq", name=f"sq{b}")
            nc.scalar.activation(sq[:], var[:], AF.Sqrt, bias=eps_t[:])
            mrs = sm.tile([GROUPS, 2], F32, tag="mrs", name=f"mrs{b}")
            nc.vector.tensor_copy(mrs[:, 0:1], gsb[:, 0:1])
            nc.vector.reciprocal(mrs[:, 1:2], sq[:])
            scl, bia = [], []
            for t in range(CT):
                bps = psA.tile([P, 2], F32, tag="attn", name=f"bps{b}_{t}")
                nc.tensor.matmul(bps[:], indTt[t], mrs[:], start=True, stop=True)
                s_ = sm.tile([P, 1], F32, tag="scl", name=f"scl{b}_{t}")
                nc.vector.tensor_tensor(s_[:], bps[:, 1:2], nwc[t], op=OP.mult)
                tmpb = sm.tile([P, 1], F32, tag="tmpb", name=f"tmpb{b}_{t}")
                nc.vector.tensor_tensor(tmpb[:], bps[:, 0:1], s_[:], op=OP.mult)
                b_ = sm.tile([P, 1], F32, tag="bia", name=f"bia{b}_{t}")
                nc.vector.tensor_tensor(b_[:], nbc[t], tmpb[:], op=OP.subtract)
                scl.append(s_)
                bia.append(b_)
            return scl, bia

        def load_xn(b, ch, scl, bia, pfx):
            xr, xn = [], []
            for t in range(CT):
                xrt = xin.tile([P, 512], F32, tag="xin", name=f"{pfx}x{b}_{ch}_{t}")
                nc.sync.dma_start(
                    xrt[:], x_d[b, t * P:(t + 1) * P, ch * 512:(ch + 1) * 512])
                xr.append(xrt)
                xnt = xnp.tile([P, 512], F32R, tag="xn", name=f"{pfx}n{b}_{ch}_{t}")
                nc.scalar.activation(xnt[:], xrt[:], AF.Identity,
                                     bias=bia[t][:], scale=scl[t][:])
                xn.append(xnt)
            return xr, xn

        def emit_v(b, ch, scl, bia):
            xr, xn2 = load_xn(b, ch, scl, bia, "v")
            vc = []
            for mt in range(CT):
                pv = psB.tile([P, 512], F32, tag="prod", name=f"pv{b}_{ch}_{mt}")
                for kt in range(CT):
                    nc.tensor.matmul(
                        pv[:],
                        wqkvT[kt][:, 2 * C + mt * P:2 * C + (mt + 1) * P],
                        xn2[kt][:],
                        start=(kt == 0), stop=(kt == CT - 1))
                vct = vch.tile([P, 512], F32R, tag="vch", name=f"vc{b}_{ch}_{mt}")
                nc.scalar.activation(vct[:], pv[:], AF.Identity,
                                     bias=vbc[mt], scale=1.0)
                vc.append(vct)
            return xr, vc

        def emit_av(b, ch, vc, eT, rs):
            ao = []
            for mt in range(CT):
                pav = psA.tile([P, 512], F32, tag="attn", name=f"pav{b}_{ch}_{mt}")
                for kt in range(CT):
                    nc.tensor.matmul(
                        pav[:],
                        eT[kt][:, mt * P:(mt + 1) * P],
                        vc[kt][:],
                        start=(kt == 0), stop=(kt == CT - 1))
                aot = aop.tile([P, 512], F32R, tag="ao", name=f"ao{b}_{ch}_{mt}")
                nc.scalar.activation(aot[:], pav[:], AF.Copy, scale=rs[mt][:])
                ao.append(aot)
            return ao

        def emit_pj(b, ch, ao, xr):
            for ot in range(CT):
                pp = psB.tile([P, 512], F32, tag="prod", name=f"pp{b}_{ch}_{ot}")
                for kt in range(CT):
                    nc.tensor.matmul(pp[:],
                                     wprojT[kt][:, ot * P:(ot + 1) * P],
                                     ao[kt][:],
                                     start=(kt == 0), stop=(kt == CT - 1))
                yt = yop.tile([P, 512], F32, tag="y", name=f"yt{b}_{ch}_{ot}")
                nc.vector.scalar_tensor_tensor(
                    out=yt[:], in0=pp[:], scalar=pbc[ot], in1=xr[ot][:],
                    op0=OP.add, op1=OP.add)
                nc.sync.dma_start(
                    y_d[b, ot * P:(ot + 1) * P, ch * 512:(ch + 1) * 512], yt[:])

        rep_cm = tc.For_i(0, repeat, 1) if repeat > 1 else nullcontext()
        with rep_cm:
          sclbia = {0: None}
          for b in range(BPC):
              if sclbia.get(b) is None:
                  ms_tiles = ms0 if (b == 0 and ms0 is not None) else [
                      stats_tile(b, t) for t in range(CT)]
                  sclbia[b] = stats_final(b, ms_tiles)
              scl, bia = sclbia[b]

              # ---- Phase Q: q/k production + attention-logit accumulation ----
              # Batch b+1's stats DMAs/DVE work are interleaved between chunks.
              apsum = [psA.tile([P, 512], F32, tag="attn", name=f"apsum{b}_{i}")
                       for i in range(CT)]
              pend_qk = None
              pend_first = True
              next_ms = []
              next_st6 = []
              for ch in range(NCH):
                  xr0, xn = load_xn(b, ch, scl, bia, "q")
                  for ns in range(4):
                      qk = qkp.tile([P, 1024], F32R, tag="qk",
                                    name=f"qk{b}_{ch}_{ns}")
                      for half in range(2):
                          hs = slice(half * 512, (half + 1) * 512)
                          ph = psB.tile([P, 512], F32, tag="prod",
                                        name=f"pqk{b}_{ch}_{ns}_{half}")
                          for kt in range(CT):
                              nc.tensor.matmul(
                                  ph[:],
                                  xn[kt][:, ns * P:(ns + 1) * P],
                                  wqkvT[kt][:, half * 512:(half + 1) * 512],
                                  start=(kt == 0), stop=(kt == CT - 1))
                          nc.vector.tensor_tensor(qk[:, hs], ph[:], qkb[:, hs],
                                                  op=OP.add)
                      if pend_qk is not None:
                          for mt in range(CT):
                              nc.tensor.matmul(
                                  apsum[mt][:],
                                  pend_qk[:, mt * P:(mt + 1) * P],
                                  pend_qk[:, 512:1024],
                                  start=pend_first, stop=False,
                                  skip_group_check=True)
                          pend_first = False
                      pend_qk = qk
                  if b + 1 < BPC:
                      # after chunk ch (0-based), do half-tile (t, half) = slot
                      t_, half = divmod(ch, 2)
                      if half == 0:
                          next_st6.append(sm.tile([P, NCH, 6], F32, tag="st6",
                                                  name=f"st6_{b + 1}_{t_}"))
                      stats_chunks(b + 1, t_, next_st6[t_],
                                   half * 4, half * 4 + 4)
                      if half == 1:
                          next_ms.append(stats_end(b + 1, t_, next_st6[t_]))
              for mt in range(CT):
                  nc.tensor.matmul(
                      apsum[mt][:],
                      pend_qk[:, mt * P:(mt + 1) * P],
                      pend_qk[:, 512:1024],
                      start=False, stop=True, skip_group_check=True)

              # v-production for chunk 0 fills the PE gap during softmax
              state = {0: {}}
              state[0]["xr"], state[0]["vc"] = emit_v(b, 0, scl, bia)

              # ---- Phase SM: softmax + probs transpose ----
              ee, rs = [], []
              for mt in range(CT):
                  lgt = lgp.tile([P, 512], F32, tag="lg", name=f"lg{b}_{mt}")
                  nc.vector.tensor_scalar_mul(lgt[:], apsum[mt][:], SCALE)
                  nmx = sm.tile([P, 1], F32, tag="nmx", name=f"nmx{b}_{mt}")
                  nc.vector.reduce_max(nmx[:], lgt[:], axis=AX.X, negate=True)
                  e_ = epool.tile([P, 512], F32, tag="e", name=f"e{b}_{mt}")
                  z_ = sm.tile([P, 1], F32, tag="z", name=f"z{b}_{mt}")
                  nc.scalar.activation(e_[:], lgt[:], AF.Exp,
                                       bias=nmx[:], scale=1.0, accum_out=z_[:])
                  r_ = sm.tile([P, 1], F32, tag="r", name=f"r{b}_{mt}")
                  nc.vector.reciprocal(r_[:], z_[:])
                  ee.append(e_)
                  rs.append(r_)
              eT = [etp.tile([P, 512], F32R, tag="eT", name=f"eT{b}_{i}")
                    for i in range(CT)]
              for mt in range(CT):
                  for dt in range(CT):
                      tp = psA.tile([P, 512], F32, tag="attn",
                                    name=f"tp{b}_{mt}_{dt}")
                      nc.tensor.transpose(tp[:, 0:P], ee[mt][:, dt * P:(dt + 1) * P],
                                          ident[:])
                      nc.vector.tensor_copy(eT[dt][:, mt * P:(mt + 1) * P],
                                            tp[:, 0:P])

              # batch b+1 stats finalize (tiny matmuls run once psA frees up)
              if b + 1 < BPC:
                  sclbia[b + 1] = stats_final(b + 1, next_ms)

              # ---- Phase AVP: v (fused) + attn@v + projection, 2-chunk skew ----
              for ch in range(1, NCH):
                  state[ch] = {}
                  state[ch]["xr"], state[ch]["vc"] = emit_v(b, ch, scl, bia)
                  state[ch - 1]["ao"] = emit_av(b, ch - 1, state[ch - 1]["vc"],
                                                eT, rs)
                  if ch >= 2:
                      emit_pj(b, ch - 2, state[ch - 2]["ao"], state[ch - 2]["xr"])
                      del state[ch - 2]
              state[NCH - 1]["ao"] = emit_av(b, NCH - 1, state[NCH - 1]["vc"],
                                             eT, rs)
              emit_pj(b, NCH - 2, state[NCH - 2]["ao"], state[NCH - 2]["xr"])
              emit_pj(b, NCH - 1, state[NCH - 1]["ao"], state[NCH - 1]["xr"])

    nc.compile()
    return nc


_NC = None


def _get_program():
    global _NC
    if _NC is None:
        _NC = build_program()
    return _NC


def make_in_maps(x, norm_w, norm_b, qkv_w, qkv_b, proj_w, proj_b):
    x = np.asarray(x, dtype=np.float32).reshape(B, C, HW)
    qkv_w = np.asarray(qkv_w, dtype=np.float32)
    proj_w = np.asarray(proj_w, dtype=np.float32)
    qkv_b = np.asarray(qkv_b, dtype=np.float32)
    nw = np.asarray(norm_w, np.float32).reshape(CT, P)
    nb = np.asarray(norm_b, np.float32).reshape(CT, P)
    vb = qkv_b[2 * C:].reshape(CT, P)
    pb = np.asarray(proj_b, np.float32).reshape(CT, P)
    cols = np.empty((P, 4 * CT), np.float32)
    for t in range(CT):
        cols[:, 4 * t + 0] = nw[t]
        cols[:, 4 * t + 1] = nb[t]
        cols[:, 4 * t + 2] = vb[t]
        cols[:, 4 * t + 3] = pb[t]
    ind = np.eye(GROUPS, dtype=np.float32)[np.arange(C) // (C // GROUPS)]  # [C, G]
    indp = np.empty((P, GROUPS * CT), np.float32)
    for t in range(CT):
        indp[:, GROUPS * t:GROUPS * (t + 1)] = ind[t * P:(t + 1) * P]
    common = {
        "wqkvT": np.ascontiguousarray(qkv_w.T),
        "wprojT": np.ascontiguousarray(proj_w.T),
        "qkb": np.ascontiguousarray(np.broadcast_to(qkv_b[:2 * C], (P, 2 * C))),
        "cols": cols,
        "indp": indp,
        "indT": np.ascontiguousarray(ind.T),
        "ident": np.eye(P, dtype=np.float32),
    }
    return [
        {"x": np.ascontiguousarray(x[i * BPC:(i + 1) * BPC]), **common}
        for i in range(NCORES)
    ]


def _wait_device(max_wait=600):
    """The axon-tunneled device can be transiently unrecoverable right after
    another process's teardown; poll with a tiny op until it responds."""
    import time
    import jax
    import jax.numpy as jnp
    t0 = time.time()
    while True:
        try:
            v = float((jnp.ones((4, 4)) @ jnp.ones((4, 4))).sum())
            assert v == 64.0
            return
        except Exception:
            if time.time() - t0 > max_wait:
                raise
            time.sleep(30)


def run(inputs, trace=False):
    import time
    from concourse.bass_utils import run_bass_kernel_spmd
    nc = _get_program()
    in_maps = make_in_maps(**inputs)
    last_err = None
    for attempt in range(3):
        try:
            if attempt > 0:
                time.sleep(60)
            _wait_device()
            r = run_bass_kernel_spmd(nc, in_maps, list(range(NCORES)), trace=trace)
            break
        except Exception as e:
            last_err = e
    else:
        raise last_err
    y = np.concatenate([r.results[i]["y"] for i in range(NCORES)], axis=0)
    return y.reshape(B, C, 64, 64), r


def kernel(**inputs):
    y, _ = run(inputs, trace=False)
    return y



# revision 4
# speedup vs baseline: 3.0331x; 3.0331x over previous
"""Trainium2 Bass kernel for nn_Attention_90752658965090.

Computes, per batch element b of x[16, 512, 64, 64]:
  xn   = GroupNorm(8 groups, eps=1e-5, affine)(x[b])            # [512, 4096]
  q,k,v = split(qkv_w @ xn + qkv_b)                             # each [512, 4096]
  attn = softmax((q @ k^T) * 512**-0.5, axis=-1)                # [512, 512]
  out  = proj_w @ (attn @ v) + proj_b                           # [512, 4096]
  y[b] = x[b] + out

Sharding: data-parallel over batch, 2 batch elements per NeuronCore on 8 cores.
Matmuls run in float32r (full-rate PE, ~12-bit mantissa, fp32 accumulate).

The final two matmuls are reassociated: proj_w @ (attn @ v) =
((proj_w . diag(1/z)) @ e) @ v where e = exp(logits - max), z = rowsum(e).
The [c,c] @ [c,c] product costs 16 MMs vs 128 for attn @ v over n, so the
per-batch MM count drops 768 -> 656; the product is emitted pre-transposed
(lhsT = e-slices) which also removes the 16 PE transposes.
"""
import sys

sys.path.insert(0, "/opt/trn_rl_repo")

import numpy as np

import concourse.bass as bass
import concourse.mybir as mybir
import concourse.tile as tile
from concourse import bacc

B, C, HW = 16, 512, 4096
NCORES = 8
BPC = B // NCORES          # batches per core
P = 128
CT = C // P                # 4 c-tiles
NCH = HW // 512            # 8 n-chunks of 512
GROUPS = 8
EPS = 1e-5
INV_N = 1.0 / (C // GROUPS)   # bn_aggr already normalizes over HW; group-combine over 64 channels
SCALE = float(C) ** -0.5

F32 = mybir.dt.float32
F32R = mybir.dt.float32r
AX = mybir.AxisListType
OP = mybir.AluOpType
AF = mybir.ActivationFunctionType


def build_program(repeat=1):
    nc = bacc.Bacc("TRN2", target_bir_lowering=False, debug=False, num_devices=NCORES)

    x_d = nc.dram_tensor("x", [BPC, C, HW], F32, kind="ExternalInput")
    y_d = nc.dram_tensor("y", [BPC, C, HW], F32, kind="ExternalOutput")
    wqkvT_d = nc.dram_tensor("wqkvT", [C, 3 * C], F32R, kind="ExternalInput")
    wprojT_d = nc.dram_tensor("wprojT", [C, C], F32, kind="ExternalInput")
    qkb_d = nc.dram_tensor("qkb", [P, 2 * C], F32, kind="ExternalInput")
    cols_d = nc.dram_tensor("cols", [P, 4 * CT], F32, kind="ExternalInput")
    indp_d = nc.dram_tensor("indp", [P, GROUPS * CT], F32, kind="ExternalInput")
    indT_d = nc.dram_tensor("indT", [GROUPS, C], F32, kind="ExternalInput")

    from contextlib import ExitStack
    with tile.TileContext(nc) as tc, ExitStack() as ctx:
        wgt = ctx.enter_context(tc.tile_pool(name="wgt", bufs=1))
        xin = ctx.enter_context(tc.tile_pool(name="xin", bufs=16))
        xpin = ctx.enter_context(tc.tile_pool(name="xpin", bufs=8))
        xnp = ctx.enter_context(tc.tile_pool(name="xnp", bufs=8))
        qkp = ctx.enter_context(tc.tile_pool(name="qkp", bufs=3))
        vch = ctx.enter_context(tc.tile_pool(name="vch", bufs=12))
        epool = ctx.enter_context(tc.tile_pool(name="epool", bufs=CT))
        patp = ctx.enter_context(tc.tile_pool(name="patp", bufs=CT))
        wpsp = ctx.enter_context(tc.tile_pool(name="wpsp", bufs=CT))
        lgp = ctx.enter_context(tc.tile_pool(name="lgp", bufs=CT))
        yop = ctx.enter_context(tc.tile_pool(name="yop", bufs=8))
        sm = ctx.enter_context(tc.tile_pool(name="sm", bufs=2 * CT))
        psA = ctx.enter_context(tc.tile_pool(name="psA", bufs=CT, space=bass.MemorySpace.PSUM))
        psB = ctx.enter_context(tc.tile_pool(name="psB", bufs=4, space=bass.MemorySpace.PSUM))

        pins = {}

        def stats_chunks(b, t, st6, ch_lo, ch_hi):
            for ch in range(ch_lo, ch_hi):
                if ch < 2:
                    xt = xpin.tile([P, 512], F32, tag="xpin", name=f"xp{b}_{t}_{ch}")
                    pins[(b, t, ch)] = xt
                else:
                    xt = xin.tile([P, 512], F32, tag="xin", name=f"xs{b}_{t}_{ch}")
                nc.sync.dma_start(
                    xt[:], x_d[b, t * P:(t + 1) * P, ch * 512:(ch + 1) * 512])
                nc.vector.bn_stats(st6[:, ch, :], xt[:])

        def stats_end(b, t, st6):
            mv = sm.tile([P, 2], F32, tag="mv", name=f"mv{b}_{t}")
            nc.vector.bn_aggr(mv[:], st6[:])
            # ms: col0 = mean, col1 = E[x^2] = var + mean^2
            ms = sm.tile([P, 2], F32, tag="ms", name=f"ms{b}_{t}")
            nc.vector.tensor_copy(ms[:, 0:1], mv[:, 0:1])
            nc.vector.scalar_tensor_tensor(
                out=ms[:, 1:2], in0=mv[:, 0:1], scalar=mv[:, 0:1],
                in1=mv[:, 1:2], op0=OP.mult, op1=OP.add)
            return ms

        def stats_tile(b, t):
            st6 = sm.tile([P, NCH, 6], F32, tag="st6", name=f"st6_{b}_{t}")
            stats_chunks(b, t, st6, 0, NCH)
            return stats_end(b, t, st6)

        # --- tiny constants first so stats_final never waits on them ---
        cols = wgt.tile([P, 4 * CT], F32, tag="cols")
        nc.sync.dma_start(cols[:], cols_d[:])
        indp = wgt.tile([P, GROUPS * CT], F32, tag="indp")
        nc.sync.dma_start(indp[:], indp_d[:])
        indT8 = wgt.tile([GROUPS, C], F32, tag="indT8")
        nc.sync.dma_start(indT8[:], indT_d[:])
        qkb = wgt.tile([P, 2 * C], F32, tag="qkb")
        nc.sync.dma_start(qkb[:], qkb_d[:])
        nwc = [cols[:, 4 * t + 0:4 * t + 1] for t in range(CT)]
        nbc = [cols[:, 4 * t + 1:4 * t + 2] for t in range(CT)]
        vbc = [cols[:, 4 * t + 2:4 * t + 3] for t in range(CT)]
        pbc = [cols[:, 4 * t + 3:4 * t + 4] for t in range(CT)]
        indt = [indp[:, GROUPS * t:GROUPS * (t + 1)] for t in range(CT)]
        indTt = [indT8[:, t * P:(t + 1) * P] for t in range(CT)]

        # --- batch-0 stats DMAs go out before the bulk weight loads so the
        # first Q matmuls (gated on stats) start as early as DMA BW allows.
        ms0 = [] if repeat == 1 else None
        if ms0 is not None:
            for t in range(CT):
                ms0.append(stats_tile(0, t))
        # qkv weights: q+k columns first (needed by the first Q matmuls),
        # v columns after.
        wqkvT = []
        for t in range(CT):
            w1 = wgt.tile([P, 3 * C], F32R, tag=f"wqkv{t}")
            nc.sync.dma_start(w1[:, 0:2 * C], wqkvT_d[t * P:(t + 1) * P, 0:2 * C])
            wqkvT.append(w1)
        for t in range(CT):
            nc.sync.dma_start(wqkvT[t][:, 2 * C:3 * C],
                              wqkvT_d[t * P:(t + 1) * P, 2 * C:3 * C])
        wprojT = []
        for t in range(CT):
            w2 = wgt.tile([P, C], F32, tag=f"wproj{t}")
            nc.sync.dma_start(w2[:], wprojT_d[t * P:(t + 1) * P, :])
            wprojT.append(w2)
        eps_t = wgt.tile([GROUPS, 1], F32, tag="eps")
        nc.vector.memset(eps_t[:], EPS)

        from contextlib import nullcontext

        def stats_final(b, ms_tiles):
            gps = psA.tile([GROUPS, 2], F32, tag="attn", name=f"gps{b}")
            for t in range(CT):
                nc.tensor.matmul(gps[:], indt[t], ms_tiles[t][:],
                                 start=(t == 0), stop=(t == CT - 1))
            gsb = sm.tile([GROUPS, 2], F32, tag="gsb", name=f"gsb{b}")
            nc.scalar.activation(gsb[:], gps[:], AF.Copy, scale=INV_N)
            m2 = sm.tile([GROUPS, 1], F32, tag="m2", name=f"m2_{b}")
            nc.vector.tensor_tensor(m2[:], gsb[:, 0:1], gsb[:, 0:1], op=OP.mult)
            var = sm.tile([GROUPS, 1], F32, tag="var", name=f"var{b}")
            nc.vector.tensor_tensor(var[:], gsb[:, 1:2], m2[:], op=OP.subtract)
            sq = sm.tile([GROUPS, 1], F32, tag="sq", name=f"sq{b}")
            nc.scalar.activation(sq[:], var[:], AF.Sqrt, bias=eps_t[:])
            mrs = sm.tile([GROUPS, 2], F32, tag="mrs", name=f"mrs{b}")
            nc.vector.tensor_copy(mrs[:, 0:1], gsb[:, 0:1])
            nc.vector.reciprocal(mrs[:, 1:2], sq[:])
            scl, bia = [], []
            for t in range(CT):
                bps = psA.tile([P, 2], F32, tag="attn", name=f"bps{b}_{t}")
                nc.tensor.matmul(bps[:], indTt[t], mrs[:], start=True, stop=True)
                s_ = sm.tile([P, 1], F32, tag="scl", name=f"scl{b}_{t}")
                nc.vector.tensor_tensor(s_[:], bps[:, 1:2], nwc[t], op=OP.mult)
                tmpb = sm.tile([P, 1], F32, tag="tmpb", name=f"tmpb{b}_{t}")
                nc.vector.tensor_tensor(tmpb[:], bps[:, 0:1], s_[:], op=OP.mult)
                b_ = sm.tile([P, 1], F32, tag="bia", name=f"bia{b}_{t}")
                nc.vector.tensor_tensor(b_[:], nbc[t], tmpb[:], op=OP.subtract)
                scl.append(s_)
                bia.append(b_)
            return scl, bia

        def load_xn(b, ch, scl, bia, pfx):
            xr, xn = [], []
            for t in range(CT):
                xrt = pins.pop((b, t, ch), None)
                if xrt is None:
                    xrt = xin.tile([P, 512], F32, tag="xin",
                                   name=f"{pfx}x{b}_{ch}_{t}")
                    nc.sync.dma_start(
                        xrt[:], x_d[b, t * P:(t + 1) * P, ch * 512:(ch + 1) * 512])
                xr.append(xrt)
                xnt = xnp.tile([P, 512], F32R, tag="xn", name=f"{pfx}n{b}_{ch}_{t}")
                nc.scalar.activation(xnt[:], xrt[:], AF.Identity,
                                     bias=bia[t][:], scale=scl[t][:])
                xn.append(xnt)
            return xr, xn

        def emit_v(b, ch, scl, bia, pspool):
            xr, xn2 = load_xn(b, ch, scl, bia, "v")
            vc = []
            for mt in range(CT):
                pv = pspool.tile([P, 512], F32,
                                 tag="attn" if pspool is psA else "prod",
                                 name=f"pv{b}_{ch}_{mt}")
                for kt in range(CT):
                    nc.tensor.matmul(
                        pv[:],
                        wqkvT[kt][:, 2 * C + mt * P:2 * C + (mt + 1) * P],
                        xn2[kt][:],
                        start=(kt == 0), stop=(kt == CT - 1))
                vct = vch.tile([P, 512], F32R, tag="vch", name=f"vc{b}_{ch}_{mt}")
                nc.scalar.activation(vct[:], pv[:], AF.Identity,
                                     bias=vbc[mt], scale=1.0)
                vc.append(vct)
            return xr, vc

        def emit_pav(b, ch, vc, pat, xr):
            # y[ot, n] = sum_dt PAT[dt][:, ot]^T @ v[dt][:, n]  (+ proj_b + x)
            for ot in range(CT):
                pp = psB.tile([P, 512], F32, tag="prod", name=f"pp{b}_{ch}_{ot}")
                for dt in range(CT):
                    nc.tensor.matmul(pp[:],
                                     pat[dt][:, ot * P:(ot + 1) * P],
                                     vc[dt][:],
                                     start=(dt == 0), stop=(dt == CT - 1))
                yt = yop.tile([P, 512], F32, tag="y", name=f"yt{b}_{ch}_{ot}")
                nc.vector.scalar_tensor_tensor(
                    out=yt[:], in0=pp[:], scalar=pbc[ot], in1=xr[ot][:],
                    op0=OP.add, op1=OP.add)
                nc.sync.dma_start(
                    y_d[b, ot * P:(ot + 1) * P, ch * 512:(ch + 1) * 512], yt[:])

        rep_cm = tc.For_i(0, repeat, 1) if repeat > 1 else nullcontext()
        with rep_cm:
          sclbia = {0: None}
          for b in range(BPC):
              if sclbia.get(b) is None:
                  ms_tiles = ms0 if (b == 0 and ms0 is not None) else [
                      stats_tile(b, t) for t in range(CT)]
                  sclbia[b] = stats_final(b, ms_tiles)
              scl, bia = sclbia[b]

              # ---- Phase Q: q/k production + attention-logit accumulation ----
              # Batch b+1's stats DMAs/DVE work are interleaved between chunks.
              apsum = [psA.tile([P, 512], F32, tag="attn", name=f"apsum{b}_{i}")
                       for i in range(CT)]
              pend_qk = None
              pend_first = True
              next_ms = []
              next_st6 = []
              for ch in range(NCH):
                  _, xn = load_xn(b, ch, scl, bia, "q")
                  for ns in range(4):
                      qk = qkp.tile([P, 1024], F32R, tag="qk",
                                    name=f"qk{b}_{ch}_{ns}")
                      for half in range(2):
                          hs = slice(half * 512, (half + 1) * 512)
                          ph = psB.tile([P, 512], F32, tag="prod",
                                        name=f"pqk{b}_{ch}_{ns}_{half}")
                          for kt in range(CT):
                              nc.tensor.matmul(
                                  ph[:],
                                  xn[kt][:, ns * P:(ns + 1) * P],
                                  wqkvT[kt][:, half * 512:(half + 1) * 512],
                                  start=(kt == 0), stop=(kt == CT - 1))
                          nc.vector.tensor_tensor(qk[:, hs], ph[:], qkb[:, hs],
                                                  op=OP.add)
                      if pend_qk is not None:
                          for mt in range(CT):
                              nc.tensor.matmul(
                                  apsum[mt][:],
                                  pend_qk[:, mt * P:(mt + 1) * P],
                                  pend_qk[:, 512:1024],
                                  start=pend_first, stop=False,
                                  skip_group_check=True)
                          pend_first = False
                      pend_qk = qk
                  if b + 1 < BPC:
                      # after chunk ch (0-based), do half-tile (t, half) = slot
                      t_, half = divmod(ch, 2)
                      if half == 0:
                          next_st6.append(sm.tile([P, NCH, 6], F32, tag="st6",
                                                  name=f"st6_{b + 1}_{t_}"))
                      stats_chunks(b + 1, t_, next_st6[t_],
                                   half * 4, half * 4 + 4)
                      if half == 1:
                          next_ms.append(stats_end(b + 1, t_, next_st6[t_]))
              for mt in range(CT):
                  nc.tensor.matmul(
                      apsum[mt][:],
                      pend_qk[:, mt * P:(mt + 1) * P],
                      pend_qk[:, 512:1024],
                      start=False, stop=True, skip_group_check=True)

              # v-production for chunks 0-1 fills the PE gap during softmax
              state = {}
              for ch in range(2):
                  state[ch] = {}
                  state[ch]["xr"], state[ch]["vc"] = emit_v(b, ch, scl, bia,
                                                            psB)

              # ---- Phase SM: softmax; then PAT = ((proj.D) @ e)^T ----
              ee, rs = [], []
              for mt in range(CT):
                  lgt = lgp.tile([P, 512], F32, tag="lg", name=f"lg{b}_{mt}")
                  nc.vector.tensor_scalar_mul(lgt[:], apsum[mt][:], SCALE)
                  nmx = sm.tile([P, 1], F32, tag="nmx", name=f"nmx{b}_{mt}")
                  nc.vector.reduce_max(nmx[:], lgt[:], axis=AX.X, negate=True)
                  e_ = epool.tile([P, 512], F32R, tag="e", name=f"e{b}_{mt}")
                  z_ = sm.tile([P, 1], F32, tag="z", name=f"z{b}_{mt}")
                  nc.scalar.activation(e_[:], lgt[:], AF.Exp,
                                       bias=nmx[:], scale=1.0, accum_out=z_[:])
                  r_ = sm.tile([P, 1], F32, tag="r", name=f"r{b}_{mt}")
                  nc.vector.reciprocal(r_[:], z_[:])
                  ee.append(e_)
                  rs.append(r_)
              # wps[kt] = wprojT[kt] * rs[kt]  (scale proj columns by 1/z)
              wps = []
              for kt in range(CT):
                  w_ = wpsp.tile([P, C], F32R, tag="wps", name=f"wps{b}_{kt}")
                  nc.scalar.activation(w_[:], wprojT[kt][:], AF.Copy,
                                       scale=rs[kt][:])
                  wps.append(w_)
              # PAT[dt][d, o] = sum_cq e[cq, d] * wps[cq, o], emitted transposed
              # via lhsT = e-slices.
              pat = []
              for dt in range(CT):
                  pap = psB.tile([P, C], F32, tag="prod", name=f"pap{b}_{dt}")
                  for kt in range(CT):
                      nc.tensor.matmul(pap[:],
                                       ee[kt][:, dt * P:(dt + 1) * P],
                                       wps[kt][:],
                                       start=(kt == 0), stop=(kt == CT - 1))
                  pt = patp.tile([P, C], F32R, tag="pat", name=f"pat{b}_{dt}")
                  nc.vector.tensor_copy(pt[:], pap[:])
                  pat.append(pt)

              # batch b+1 stats finalize (tiny matmuls run once psA frees up)
              if b + 1 < BPC:
                  sclbia[b + 1] = stats_final(b + 1, next_ms)

              # ---- Phase AVP: v (fused) + ((proj.D)e)^T @ v, 1-chunk skew ----
              for ch in range(2, NCH):
                  state[ch] = {}
                  state[ch]["xr"], state[ch]["vc"] = emit_v(b, ch, scl, bia,
                                                            psA)
                  emit_pav(b, ch - 2, state[ch - 2]["vc"], pat,
                           state[ch - 2]["xr"])
                  del state[ch - 2]
              emit_pav(b, NCH - 2, state[NCH - 2]["vc"], pat, state[NCH - 2]["xr"])
              emit_pav(b, NCH - 1, state[NCH - 1]["vc"], pat, state[NCH - 1]["xr"])

    nc.compile()
    return nc


_NC = None


def _get_program():
    global _NC
    if _NC is None:
        _NC = build_program()
    return _NC


def make_in_maps(x, norm_w, norm_b, qkv_w, qkv_b, proj_w, proj_b):
    x = np.asarray(x, dtype=np.float32).reshape(B, C, HW)
    qkv_w = np.asarray(qkv_w, dtype=np.float32)
    proj_w = np.asarray(proj_w, dtype=np.float32)
    qkv_b = np.asarray(qkv_b, dtype=np.float32)
    nw = np.asarray(norm_w, np.float32).reshape(CT, P)
    nb = np.asarray(norm_b, np.float32).reshape(CT, P)
    vb = qkv_b[2 * C:].reshape(CT, P)
    pb = np.asarray(proj_b, np.float32).reshape(CT, P)
    cols = np.empty((P, 4 * CT), np.float32)
    for t in range(CT):
        cols[:, 4 * t + 0] = nw[t]
        cols[:, 4 * t + 1] = nb[t]
        cols[:, 4 * t + 2] = vb[t]
        cols[:, 4 * t + 3] = pb[t]
    ind = np.eye(GROUPS, dtype=np.float32)[np.arange(C) // (C // GROUPS)]  # [C, G]
    indp = np.empty((P, GROUPS * CT), np.float32)
    for t in range(CT):
        indp[:, GROUPS * t:GROUPS * (t + 1)] = ind[t * P:(t + 1) * P]
    common = {
        "wqkvT": np.ascontiguousarray(qkv_w.T),
        "wprojT": np.ascontiguousarray(proj_w.T),
        "qkb": np.ascontiguousarray(np.broadcast_to(qkv_b[:2 * C], (P, 2 * C))),
        "cols": cols,
        "indp": indp,
        "indT": np.ascontiguousarray(ind.T),
    }
    return [
        {"x": np.ascontiguousarray(x[i * BPC:(i + 1) * BPC]), **common}
        for i in range(NCORES)
    ]


def _wait_device(max_wait=600):
    """The axon-tunneled device can be transiently unrecoverable right after
    another process's teardown; poll with a tiny op until it responds."""
    import time
    import jax
    import jax.numpy as jnp
    t0 = time.time()
    while True:
        try:
            v = float((jnp.ones((4, 4)) @ jnp.ones((4, 4))).sum())
            assert v == 64.0
            return
        except Exception:
            if time.time() - t0 > max_wait:
                raise
            time.sleep(30)


def run(inputs, trace=False):
    import time
    from concourse.bass_utils import run_bass_kernel_spmd
    nc = _get_program()
    in_maps = make_in_maps(**inputs)
    last_err = None
    for attempt in range(3):
        try:
            if attempt > 0:
                time.sleep(60)
            _wait_device()
            r = run_bass_kernel_spmd(nc, in_maps, list(range(NCORES)), trace=trace)
            break
        except Exception as e:
            last_err = e
    else:
        raise last_err
    y = np.concatenate([r.results[i]["y"] for i in range(NCORES)], axis=0)
    return y.reshape(B, C, 64, 64), r


def kernel(**inputs):
    y, _ = run(inputs, trace=False)
    return y
